# revision 16
# baseline (speedup 1.0000x reference)
"""DWA-CNN (DTW-aligned CNN) Trainium2 kernel.

Problem: x[32,2048,128], w[3,128,8], b[8] -> out[32,2046,8]
out[b,p,f] = relu(b[f] + sum of dots along the DTW-optimal path between
window x[b,p:p+3,:] and filter w[:,:,f]).

Strategy (8 cores, pure data parallel over batch, 4 batches/core):
- Host pre-transposes x to [C=128, 8192] per core, split into an exact
  bf16 hi/lo pair (xh + xl == x in fp32), and prepares -2w in a matching
  bf16 hi/lo pair plus fp32 row norms nS and weight norms nW.
- Device: Gm = -2*dots via 4 bf16 matmuls per 512-position block
  (wh*xh + wh*xl + wl*xh + wl*xl accumulated in fp32 PSUM — numerically
  equivalent to an fp32 matmul, much faster on the PE).
- PSUM evacuated by ScalarE into a [24, 8192] staging tile; a DRAM
  bounce (6 writes + 2 reads + 2 halo DMAs, split by halves for overlap)
  rewrites rows into the scattered [(q*8+f) partition, position] layout
  (DRAM APs absorb the partition remap that SBUF APs cannot express).
- E = Gm + nS on VectorE; D = sqrt(E + nW) on ScalarE; the K=3 DTW cost
  DP and path backtrack are fully unrolled into elementwise
  min/add/cmp/copy_predicated ops on VectorE, with independent op pairs
  fused into single two-window instructions (hand-built strided APs).
  Selects replicate the reference argmin tie-break (diag > left > up).
- Final relu(-0.5*acc + b) on ScalarE; result [128,512] DMAed out; host
  re-assembles [32,2046,8].
"""
import numpy as np

B, T, C, K, F = 32, 2048, 128, 3, 8
P = T - K + 1            # 2046
NCORES = 8
NB = B // NCORES         # batches per core
TL = NB * T              # 8192 positions per core
FD = 512
NQ = TL // FD            # 16 q blocks
NXCH = 4                 # x DMA chunks per half
JW = FD + 4              # per-j stride in the concatenated arrays

_cache = {}


def _build_program():
    import concourse.tile as tile
    from concourse import bacc, mybir

    f32 = mybir.dt.float32
    bf16 = mybir.dt.bfloat16
    u32 = mybir.dt.uint32
    Alu = mybir.AluOpType
    Act = mybir.ActivationFunctionType

    nc = bacc.Bacc(
        "TRN2",
        target_bir_lowering=False,
        debug=False,
        enable_asserts=False,
        num_devices=NCORES,
    )

    xh = nc.dram_tensor("xh", [C, TL], bf16, kind="ExternalInput").ap()
    xl = nc.dram_tensor("xl", [C, TL], bf16, kind="ExternalInput").ap()
    wcat = nc.dram_tensor("wcat", [C, 48], bf16, kind="ExternalInput").ap()
    nsr = nc.dram_tensor("nsr", [C, 3 * JW], f32, kind="ExternalInput").ap()
    biasc = nc.dram_tensor("biasc", [C, 4], f32, kind="ExternalInput").ap()
    res = nc.dram_tensor("res", [C, FD], f32, kind="ExternalOutput").ap()

    from contextlib import ExitStack

    with tile.TileContext(nc) as tc, ExitStack() as ctx:
        const = ctx.enter_context(tc.tile_pool(name="const", bufs=1))
        xin = ctx.enter_context(tc.tile_pool(name="xin", bufs=1))
        psum = ctx.enter_context(tc.tile_pool(name="psum", bufs=2, space="PSUM"))
        stage = ctx.enter_context(tc.tile_pool(name="stage", bufs=1))
        arrs = ctx.enter_context(tc.tile_pool(name="arrs", bufs=1))
        work = ctx.enter_context(tc.tile_pool(name="work", bufs=1))
        dramp = ctx.enter_context(
            tc.tile_pool(name="dramp", bufs=1, space="DRAM"))
        scat_h = [dramp.tile([K, 64, FD], f32, tag=f"scat{h}", name=f"scat{h}")
                  for h in range(2)]

        dmaengs = [nc.sync, nc.gpsimd]
        _dmac = [0]

        def dma(dst, src):
            e = dmaengs[_dmac[0] % len(dmaengs)]
            _dmac[0] += 1
            e.dma_start(dst, src)

        wcat_sb = const.tile([C, 48], bf16, tag="wcat")
        bias_sb = const.tile([C, 4], f32, tag="bias")
        nsr_sb = const.tile([C, 3 * JW], f32, tag="nsr")
        nc.sync.dma_start(wcat_sb[:], wcat)
        nc.sync.dma_start(bias_sb[:], biasc)
        nc.gpsimd.dma_start(nsr_sb[:], nsr)

        CH = TL // NXCH
        xh_t = []
        xl_t = []
        for i in range(NXCH):
            th = xin.tile([C, CH], bf16, tag=f"xh{i}", name=f"xh{i}")
            tl_ = xin.tile([C, CH], bf16, tag=f"xl{i}", name=f"xl{i}")
            dma(th[:], xh[:, i * CH:(i + 1) * CH])
            dma(tl_[:], xl[:, i * CH:(i + 1) * CH])
            xh_t.append(th)
            xl_t.append(tl_)

        # staging for all 16 q blocks (fp32 rows j*8+f)
        stg = stage.tile([24, TL], f32, tag="stg")

        wh = wcat_sb[:, 0:24]
        wl = wcat_sb[:, 24:48]
        for quad in range(4):
            ps = psum.tile([24, 4 * FD], f32, tag="ps", name=f"ps{quad}")
            for k in range(4):
                xs = slice(k * FD, (k + 1) * FD)
                po = slice(k * FD, (k + 1) * FD)
                nc.tensor.matmul(ps[:, po], wh, xh_t[quad][:, xs],
                                 start=True, stop=False)
                nc.tensor.matmul(ps[:, po], wh, xl_t[quad][:, xs],
                                 start=False, stop=False)
            for k in range(4):
                xs = slice(k * FD, (k + 1) * FD)
                po = slice(k * FD, (k + 1) * FD)
                nc.tensor.matmul(ps[:, po], wl, xh_t[quad][:, xs],
                                 start=False, stop=False)
                nc.tensor.matmul(ps[:, po], wl, xl_t[quad][:, xs],
                                 start=False, stop=True)
            nc.scalar.copy(stg[:, quad * 4 * FD:(quad + 1) * 4 * FD], ps[:])

        # arrays in scattered layout [partition q*8+f, j*JW + pos]
        Gm_all = arrs.tile([C, K * JW], f32, tag="gm")
        Eg_all = arrs.tile([C, K * JW], f32, tag="eg")
        Dj_all = arrs.tile([C, K * JW], f32, tag="dj")

        # bounce through DRAM, split by halves (q0-7 / q8-15) for overlap
        for h in range(2):
            for j in range(K):
                dst = scat_h[h][j].rearrange("(q f) e -> f q e", q=8)
                src = stg[j * 8:(j + 1) * 8,
                          h * (TL // 2):(h + 1) * (TL // 2)].rearrange(
                    "f (q e) -> f q e", q=8)
                dma(dst, src)
        for h in range(2):
            dst = Gm_all[h * 64:(h + 1) * 64, :].rearrange(
                "p (j e) -> p j e", j=K)[:, :, 0:FD]
            dma(dst, scat_h[h].rearrange("j p e -> p j e"))
        # halo tails default 0 for partitions >= 96 (only 120..127 kept)
        for j in range(K):
            nc.vector.memset(Gm_all[96:128, j * JW + FD:j * JW + FD + 2], 0.0)
        # halo: partition i cols 512:514 <- scat row i+8 cols 0:2
        dma(Gm_all[0:56, :].rearrange("p (j e) -> p j e", j=K)[:, :, FD:FD + 2],
            scat_h[0].rearrange("j p e -> p j e")[8:64, :, 0:2])
        dma(Gm_all[56:120, :].rearrange(
            "p (j e) -> p j e", j=K)[:, :, FD:FD + 2],
            scat_h[1].rearrange("j p e -> p j e")[:, :, 0:2])

        V = nc.vector

        # E = Gm + nS per j (pipelines with the sqrt on ScalarE)
        for j in range(K):
            sl = slice(j * JW, j * JW + FD + 2)
            V.tensor_tensor(Eg_all[:, sl], Gm_all[:, sl], nsr_sb[:, sl],
                            Alu.add)
            nc.scalar.activation(Dj_all[:, sl], Eg_all[:, sl], Act.Sqrt,
                                 bias=bias_sb[:, j:j + 1])

        # mega work tiles with manual slot layout
        NW = 27
        W = work.tile([C, NW * FD], f32, tag="W")
        M = work.tile([C, 8 * FD], u32, tag="M")
        (S_c12, S_c21, S_c13, S_c31, S_mbcA, S_mnA, S_c22, S_mbcB, S_mbcC,
         S_mnB, S_mnC, S_c23, S_c32, S_mbcD, S_s10, S_s01, S_s20, S_s02,
         S_X1, S_U, S_X2, S_X3, S_XV1, S_XV2, S_X4, S_ACCM, S_res) = range(NW)
        M_KA, M_KB, M_KC, M_KD, M_LA, M_LB, M_LC, M_LD = range(8)

        def w1(s):
            return W[:, s * FD:(s + 1) * FD]

        def w2(s):
            return W[:, s * FD:(s + 2) * FD]

        def win2(ap2d, off_a, off_b, n=FD):
            v = ap2d[:, off_a:off_a + n].unsqueeze(1)
            v.ap[1] = [off_b - off_a, 2]
            return v

        def wwin(sa, sb):
            return win2(W[:], sa * FD, sb * FD)

        def mwin(sa, sb):
            return win2(M[:], sa * FD, sb * FD)

        def m1(s):
            return M[:, s * FD:(s + 1) * FD]

        def dwin(ia, ja, ib, jb):
            return win2(Dj_all[:], ja * JW + ia, jb * JW + ib)

        def gwin(ia, ja, ib, jb):
            return win2(Gm_all[:], ja * JW + ia, jb * JW + ib)

        def dd(i, j):
            return Dj_all[:, j * JW + i:j * JW + i + FD]

        def gg(i, j):
            return Gm_all[:, j * JW + i:j * JW + i + FD]

        def r2(s):
            # [128, 2, 512] view of two adjacent W slots
            return W[:, s * FD:(s + 2) * FD].rearrange("p (r e) -> p r e", r=2)

        def rm2(s):
            return M[:, s * FD:(s + 2) * FD].rearrange("p (r e) -> p r e", r=2)

        TT = V.tensor_tensor
        PT = nc.gpsimd.tensor_tensor
        # [c12|c21] = [D(0,1)|D(1,0)] + c11(x2)
        TT(r2(S_c12), dwin(0, 1, 1, 0), dwin(0, 0, 0, 0), Alu.add)
        # [c13|c31] = [D(0,2)|D(2,0)] + [c12|c21]
        TT(r2(S_c13), dwin(0, 2, 2, 0), r2(S_c12), Alu.add)
        # [s10|s01] = [g(1,0)|g(0,1)] + g00(x2)
        TT(r2(S_s10), gwin(1, 0, 0, 1), gwin(0, 0, 0, 0), Alu.add)
        # [s20|s02] = [g(2,0)|g(0,2)] + [s10|s01]
        TT(r2(S_s20), gwin(2, 0, 0, 2), r2(S_s10), Alu.add)

        TT(w1(S_mbcA), w1(S_c21), w1(S_c12), Alu.min)
        TT(w1(S_mnA), dd(0, 0), w1(S_mbcA), Alu.min)
        TT(w1(S_c22), dd(1, 1), w1(S_mnA), Alu.add)
        # [mbcB|mbcC] = min([c22|c31], [c13|c22])
        TT(r2(S_mbcB), wwin(S_c22, S_c31), wwin(S_c13, S_c22), Alu.min)
        # [mnB|mnC] = min([c12|c21], [mbcB|mbcC])
        TT(r2(S_mnB), r2(S_c12), r2(S_mbcB), Alu.min)
        # [c23|c32] = [D(1,2)|D(2,1)] + [mnB|mnC]
        TT(r2(S_c23), dwin(1, 2, 2, 1), r2(S_mnB), Alu.add)
        TT(w1(S_mbcD), w1(S_c32), w1(S_c23), Alu.min)

        TT(m1(M_KA), dd(0, 0), w1(S_mbcA), Alu.is_le)
        # [KB|KC] = [c12|c21] <= [mbcB|mbcC]
        TT(rm2(M_KB), r2(S_c12), r2(S_mbcB), Alu.is_le)
        TT(m1(M_KD), w1(S_c22), w1(S_mbcD), Alu.is_le)
        # [LA|LB] = [c21|c22] <= [c12|c13]
        TT(rm2(M_LA), wwin(S_c21, S_c22), wwin(S_c12, S_c13), Alu.is_le)
        # [LC|LD] = [c31|c32] <= [c22|c23]
        TT(rm2(M_LC), wwin(S_c31, S_c32), wwin(S_c22, S_c23), Alu.is_le)

        CP = V.copy_predicated
        nc.scalar.copy(w1(S_X1), w1(S_s01))
        CP(w1(S_X1), m1(M_LA), w1(S_s10))
        CP(w1(S_X1), m1(M_KA), gg(0, 0))
        TT(w1(S_U), gg(1, 1), w1(S_X1), Alu.add)

        nc.scalar.copy(w1(S_X2), w1(S_U))
        nc.scalar.copy(w1(S_X3), w1(S_s02))
        # [X2|X3] where [LC|LB]: [s20|U]
        CP(r2(S_X2), mwin(M_LC, M_LB), wwin(S_s20, S_U))
        # [X2|X3] where [KC|KB]: [s10|s01]
        CP(r2(S_X2), mwin(M_KC, M_KB), r2(S_s10))
        # [XV1|XV2] = [g(2,1)|g(1,2)] + [X2|X3]
        TT(r2(S_XV1), gwin(2, 1, 1, 2), r2(S_X2), Alu.add)

        nc.scalar.copy(w1(S_X4), w1(S_XV2))
        CP(w1(S_X4), m1(M_LD), w1(S_XV1))
        CP(w1(S_X4), m1(M_KD), w1(S_U))
        TT(w1(S_ACCM), gg(2, 2), w1(S_X4), Alu.add)

        nc.scalar.activation(
            w1(S_res), w1(S_ACCM), Act.Relu, bias=bias_sb[:, 3:4], scale=-0.5)
        nc.sync.dma_start(res, w1(S_res))

    nc.compile()
    return nc


def _host_prep(x, w, b):
    """Build per-core input maps."""
    import ml_dtypes

    x = np.ascontiguousarray(np.asarray(x, np.float32))
    w = np.asarray(w, np.float32)
    b = np.asarray(b, np.float32)

    w2m = np.zeros((C, 24), np.float32)
    for j in range(K):
        for f in range(F):
            w2m[:, j * 8 + f] = -2.0 * w[j, :, f]
    wh = w2m.astype(ml_dtypes.bfloat16)
    wlo = (w2m - wh.astype(np.float32)).astype(ml_dtypes.bfloat16)
    wcat = np.concatenate([wh, wlo], axis=1)              # [C, 48] bf16

    nW = (w ** 2).sum(1)                                  # [K, F]
    biasc = np.zeros((C, 4), np.float32)
    for q in range(NQ):
        for f in range(F):
            for j in range(K):
                biasc[q * 8 + f, j] = nW[j, f]
            biasc[q * 8 + f, 3] = b[f]

    in_maps = []
    for r in range(NCORES):
        x4 = x[r * NB:(r + 1) * NB]                       # [NB,T,C]
        flat = x4.reshape(TL, C)
        xT = np.ascontiguousarray(flat.T)                 # [C, TL] fp32
        xhh = xT.astype(ml_dtypes.bfloat16)
        xll = (xT - xhh.astype(np.float32)).astype(ml_dtypes.bfloat16)
        nS = np.einsum("tc,tc->t", flat, flat).astype(np.float32)
        nsr = np.ones((C, 3 * JW), np.float32)
        block = np.ones((NQ * 8, JW), np.float32)
        for q in range(NQ):
            lo = q * FD
            hi = min(TL, lo + FD + 2)
            block[q * 8:(q + 1) * 8, 0:hi - lo] = nS[lo:hi][None, :]
        for j in range(K):
            nsr[:, j * JW:(j + 1) * JW] = block
        in_maps.append({
            "xh": xhh, "xl": xll, "wcat": wcat, "nsr": nsr, "biasc": biasc,
        })
    return in_maps


def _assemble(results):
    out = np.empty((B, P, F), np.float32)
    for r in range(NCORES):
        resr = results[r]["res"]                          # [128, 512]
        arr = resr.reshape(NQ, 8, FD)                     # [q, f, p_lo]
        for f in range(F):
            series = arr[:, f, :].reshape(TL).reshape(NB, T)
            out[r * NB:(r + 1) * NB, :, f] = series[:, :P]
    return out


def kernel(x, w, b):
    from concourse.bass_utils import run_bass_kernel_spmd

    if "nc" not in _cache:
        _cache["nc"] = _build_program()
    nc = _cache["nc"]
    in_maps = _host_prep(x, w, b)
    out = run_bass_kernel_spmd(nc, in_maps, core_ids=list(range(NCORES)))
    return _assemble(out.results)


if __name__ == "__main__":
    rng = np.random.default_rng(0)
    x = rng.standard_normal((B, T, C), dtype=np.float32)
    w = (rng.standard_normal((K, C, F)) * 0.1).astype(np.float32)
    b = np.zeros((F,), np.float32)
    o = kernel(x, w, b)
    print("kernel ran, out shape", o.shape, float(np.abs(o).sum()))


# revision 17
# speedup vs baseline: 1.0683x; 1.0683x over previous
"""DWA-CNN (DTW-aligned CNN) Trainium2 kernel.

Problem: x[32,2048,128], w[3,128,8], b[8] -> out[32,2046,8]
out[b,p,f] = relu(b[f] + sum of dots along the DTW-optimal path between
window x[b,p:p+3,:] and filter w[:,:,f]).

Strategy (8 cores, pure data parallel over batch, 4 batches/core):
- Host pre-transposes x to [C=128, 8192] per core, split into an exact
  bf16 hi/lo pair (xh + xl == x in fp32), and prepares -2w in a matching
  bf16 hi/lo pair plus fp32 row norms nS and weight norms nW.
- Device: Gm = -2*dots via 4 bf16 matmuls per 512-position block
  (wh*xh + wh*xl + wl*xh + wl*xl accumulated in fp32 PSUM — numerically
  equivalent to an fp32 matmul, much faster on the PE).
- PSUM evacuated by ScalarE into a [24, 8192] staging tile; a DRAM
  bounce (6 writes + 2 reads + 2 halo DMAs, split by halves for overlap)
  rewrites rows into the scattered [(q*8+f) partition, position] layout
  (DRAM APs absorb the partition remap that SBUF APs cannot express).
- E = Gm + nS on VectorE; D = sqrt(E + nW) on ScalarE; the K=3 DTW cost
  DP and path backtrack are fully unrolled into elementwise
  min/add/cmp/copy_predicated ops on VectorE, with independent op pairs
  fused into single two-window instructions (hand-built strided APs).
  Selects replicate the reference argmin tie-break (diag > left > up).
- Final relu(-0.5*acc + b) on ScalarE; result [128,512] DMAed out; host
  re-assembles [32,2046,8].
"""
import numpy as np

B, T, C, K, F = 32, 2048, 128, 3, 8
P = T - K + 1            # 2046
NCORES = 8
NB = B // NCORES         # batches per core
TL = NB * T              # 8192 positions per core
FD = 512
NQ = TL // FD            # 16 q blocks
NXCH = 4                 # x DMA chunks per half
JW = FD + 4              # per-j stride in the concatenated arrays

_cache = {}


def _build_program():
    import concourse.tile as tile
    from concourse import bacc, mybir

    f32 = mybir.dt.float32
    bf16 = mybir.dt.bfloat16
    u32 = mybir.dt.uint32
    Alu = mybir.AluOpType
    Act = mybir.ActivationFunctionType

    nc = bacc.Bacc(
        "TRN2",
        target_bir_lowering=False,
        debug=False,
        enable_asserts=False,
        num_devices=NCORES,
    )

    xh = nc.dram_tensor("xh", [C, TL], bf16, kind="ExternalInput").ap()
    xl = nc.dram_tensor("xl", [C, TL], bf16, kind="ExternalInput").ap()
    wcat = nc.dram_tensor("wcat", [C, 48], bf16, kind="ExternalInput").ap()
    nsr = nc.dram_tensor("nsr", [C, 3 * JW], f32, kind="ExternalInput").ap()
    biasc = nc.dram_tensor("biasc", [C, 4], f32, kind="ExternalInput").ap()
    res = nc.dram_tensor("res", [C, FD], f32, kind="ExternalOutput").ap()

    from contextlib import ExitStack

    with tile.TileContext(nc) as tc, ExitStack() as ctx:
        const = ctx.enter_context(tc.tile_pool(name="const", bufs=1))
        xin = ctx.enter_context(tc.tile_pool(name="xin", bufs=1))
        psum = ctx.enter_context(tc.tile_pool(name="psum", bufs=2, space="PSUM"))
        stage = ctx.enter_context(tc.tile_pool(name="stage", bufs=1))
        arrs = ctx.enter_context(tc.tile_pool(name="arrs", bufs=1))
        work = ctx.enter_context(tc.tile_pool(name="work", bufs=1))
        dramp = ctx.enter_context(
            tc.tile_pool(name="dramp", bufs=1, space="DRAM"))
        scat_h = [dramp.tile([K, 64, FD], f32, tag=f"scat{h}", name=f"scat{h}")
                  for h in range(2)]

        dmaengs = [nc.sync, nc.gpsimd]
        _dmac = [0]

        def dma(dst, src):
            e = dmaengs[_dmac[0] % len(dmaengs)]
            _dmac[0] += 1
            e.dma_start(dst, src)

        wcat_sb = const.tile([C, 48], bf16, tag="wcat")
        bias_sb = const.tile([C, 4], f32, tag="bias")
        nsr_sb = const.tile([C, 3 * JW], f32, tag="nsr")
        nc.sync.dma_start(wcat_sb[:], wcat)

        CH = TL // NXCH  # one chunk tile per quad
        xh_t = []
        xl_t = []
        for i in range(NXCH):
            th = xin.tile([C, CH], bf16, tag=f"xh{i}", name=f"xh{i}")
            tl_ = xin.tile([C, CH], bf16, tag=f"xl{i}", name=f"xl{i}")
            # two half-chunk DMAs each for queue parallelism
            H2 = CH // 2
            for hh in range(2):
                dma(th[:, hh * H2:(hh + 1) * H2],
                    xh[:, i * CH + hh * H2:i * CH + (hh + 1) * H2])
                dma(tl_[:, hh * H2:(hh + 1) * H2],
                    xl[:, i * CH + hh * H2:i * CH + (hh + 1) * H2])
            xh_t.append(th)
            xl_t.append(tl_)
        nc.sync.dma_start(bias_sb[:], biasc)
        nc.gpsimd.dma_start(nsr_sb[:], nsr)

        # staging for all 16 q blocks (fp32 rows j*8+f)
        stg = stage.tile([24, TL], f32, tag="stg")

        wh = wcat_sb[:, 0:24]
        wl = wcat_sb[:, 24:48]
        for quad in range(4):
            ps = psum.tile([24, 4 * FD], f32, tag="ps", name=f"ps{quad}")
            for k in range(4):
                xs = slice(k * FD, (k + 1) * FD)
                po = slice(k * FD, (k + 1) * FD)
                nc.tensor.matmul(ps[:, po], wh, xh_t[quad][:, xs],
                                 start=True, stop=False)
                nc.tensor.matmul(ps[:, po], wh, xl_t[quad][:, xs],
                                 start=False, stop=False)
            for k in range(4):
                xs = slice(k * FD, (k + 1) * FD)
                po = slice(k * FD, (k + 1) * FD)
                nc.tensor.matmul(ps[:, po], wl, xh_t[quad][:, xs],
                                 start=False, stop=True)
            nc.scalar.copy(stg[:, quad * 4 * FD:(quad + 1) * 4 * FD], ps[:])

        # arrays in scattered layout [partition q*8+f, j*JW + pos]
        Gm_all = arrs.tile([C, K * JW], f32, tag="gm")
        Eg_all = arrs.tile([C, K * JW], f32, tag="eg")
        Dj_all = arrs.tile([C, K * JW], f32, tag="dj")

        # halo tails default 0 for partitions >= 96 (only 120..127 kept)
        for j in range(K):
            nc.vector.memset(Gm_all[96:128, j * JW + FD:j * JW + FD + 2], 0.0)
        # bounce through DRAM, split by halves (q0-7 / q8-15) for overlap;
        # writes on sync, reads + halo on gpsimd (no engine-order coupling)
        for h in range(2):
            for j in range(K):
                dst = scat_h[h][j].rearrange("(q f) e -> f q e", q=8)
                src = stg[j * 8:(j + 1) * 8,
                          h * (TL // 2):(h + 1) * (TL // 2)].rearrange(
                    "f (q e) -> f q e", q=8)
                nc.sync.dma_start(dst, src)
            rdst = Gm_all[h * 64:(h + 1) * 64, :].rearrange(
                "p (j e) -> p j e", j=K)[:, :, 0:FD]
            nc.gpsimd.dma_start(rdst, scat_h[h].rearrange("j p e -> p j e"))
            if h == 0:
                nc.gpsimd.dma_start(
                    Gm_all[0:56, :].rearrange(
                        "p (j e) -> p j e", j=K)[:, :, FD:FD + 2],
                    scat_h[0].rearrange("j p e -> p j e")[8:64, :, 0:2])
            else:
                nc.gpsimd.dma_start(
                    Gm_all[56:120, :].rearrange(
                        "p (j e) -> p j e", j=K)[:, :, FD:FD + 2],
                    scat_h[1].rearrange("j p e -> p j e")[:, :, 0:2])

        V = nc.vector

        # E = Gm + nS per j (pipelines with the sqrt on ScalarE)
        for j in range(K):
            sl = slice(j * JW, j * JW + FD + 2)
            V.tensor_tensor(Eg_all[:, sl], Gm_all[:, sl], nsr_sb[:, sl],
                            Alu.add)
            nc.scalar.activation(Dj_all[:, sl], Eg_all[:, sl], Act.Sqrt,
                                 bias=bias_sb[:, j:j + 1])

        # mega work tiles with manual slot layout
        NW = 27
        W = work.tile([C, NW * FD], f32, tag="W")
        M = work.tile([C, 8 * FD], u32, tag="M")
        (S_c12, S_c21, S_c13, S_c31, S_mbcA, S_mnA, S_c22, S_mbcB, S_mbcC,
         S_mnB, S_mnC, S_c23, S_c32, S_mbcD, S_s10, S_s01, S_s20, S_s02,
         S_X1, S_U, S_X2, S_X3, S_XV1, S_XV2, S_X4, S_ACCM, S_res) = range(NW)
        M_KA, M_KB, M_KC, M_KD, M_LA, M_LB, M_LC, M_LD = range(8)

        def w1(s):
            return W[:, s * FD:(s + 1) * FD]

        def w2(s):
            return W[:, s * FD:(s + 2) * FD]

        def win2(ap2d, off_a, off_b, n=FD):
            v = ap2d[:, off_a:off_a + n].unsqueeze(1)
            v.ap[1] = [off_b - off_a, 2]
            return v

        def wwin(sa, sb):
            return win2(W[:], sa * FD, sb * FD)

        def mwin(sa, sb):
            return win2(M[:], sa * FD, sb * FD)

        def m1(s):
            return M[:, s * FD:(s + 1) * FD]

        def dwin(ia, ja, ib, jb):
            return win2(Dj_all[:], ja * JW + ia, jb * JW + ib)

        def gwin(ia, ja, ib, jb):
            return win2(Gm_all[:], ja * JW + ia, jb * JW + ib)

        def dd(i, j):
            return Dj_all[:, j * JW + i:j * JW + i + FD]

        def gg(i, j):
            return Gm_all[:, j * JW + i:j * JW + i + FD]

        def r2(s):
            # [128, 2, 512] view of two adjacent W slots
            return W[:, s * FD:(s + 2) * FD].rearrange("p (r e) -> p r e", r=2)

        def rm2(s):
            return M[:, s * FD:(s + 2) * FD].rearrange("p (r e) -> p r e", r=2)

        TT = V.tensor_tensor
        PT = nc.gpsimd.tensor_tensor
        # [c12|c21] = [D(0,1)|D(1,0)] + c11(x2)
        TT(r2(S_c12), dwin(0, 1, 1, 0), dwin(0, 0, 0, 0), Alu.add)
        # [c13|c31] = [D(0,2)|D(2,0)] + [c12|c21]
        TT(r2(S_c13), dwin(0, 2, 2, 0), r2(S_c12), Alu.add)
        # [s10|s01] = [g(1,0)|g(0,1)] + g00(x2)
        TT(r2(S_s10), gwin(1, 0, 0, 1), gwin(0, 0, 0, 0), Alu.add)
        # [s20|s02] = [g(2,0)|g(0,2)] + [s10|s01]
        TT(r2(S_s20), gwin(2, 0, 0, 2), r2(S_s10), Alu.add)

        TT(w1(S_mbcA), w1(S_c21), w1(S_c12), Alu.min)
        TT(w1(S_mnA), dd(0, 0), w1(S_mbcA), Alu.min)
        TT(w1(S_c22), dd(1, 1), w1(S_mnA), Alu.add)
        # [mbcB|mbcC] = min([c22|c31], [c13|c22])
        TT(r2(S_mbcB), wwin(S_c22, S_c31), wwin(S_c13, S_c22), Alu.min)
        # [mnB|mnC] = min([c12|c21], [mbcB|mbcC])
        TT(r2(S_mnB), r2(S_c12), r2(S_mbcB), Alu.min)
        # [c23|c32] = [D(1,2)|D(2,1)] + [mnB|mnC]
        TT(r2(S_c23), dwin(1, 2, 2, 1), r2(S_mnB), Alu.add)
        TT(w1(S_mbcD), w1(S_c32), w1(S_c23), Alu.min)

        TT(m1(M_KA), dd(0, 0), w1(S_mbcA), Alu.is_le)
        # [KB|KC] = [c12|c21] <= [mbcB|mbcC]
        TT(rm2(M_KB), r2(S_c12), r2(S_mbcB), Alu.is_le)
        TT(m1(M_KD), w1(S_c22), w1(S_mbcD), Alu.is_le)
        # [LA|LB] = [c21|c22] <= [c12|c13]
        TT(rm2(M_LA), wwin(S_c21, S_c22), wwin(S_c12, S_c13), Alu.is_le)
        # [LC|LD] = [c31|c32] <= [c22|c23]
        TT(rm2(M_LC), wwin(S_c31, S_c32), wwin(S_c22, S_c23), Alu.is_le)

        CP = V.copy_predicated
        nc.scalar.copy(w1(S_X1), w1(S_s01))
        CP(w1(S_X1), m1(M_LA), w1(S_s10))
        CP(w1(S_X1), m1(M_KA), gg(0, 0))
        TT(w1(S_U), gg(1, 1), w1(S_X1), Alu.add)

        nc.scalar.copy(w1(S_X2), w1(S_U))
        nc.scalar.copy(w1(S_X3), w1(S_s02))
        # [X2|X3] where [LC|LB]: [s20|U]
        CP(r2(S_X2), mwin(M_LC, M_LB), wwin(S_s20, S_U))
        # [X2|X3] where [KC|KB]: [s10|s01]
        CP(r2(S_X2), mwin(M_KC, M_KB), r2(S_s10))
        # [XV1|XV2] = [g(2,1)|g(1,2)] + [X2|X3]
        TT(r2(S_XV1), gwin(2, 1, 1, 2), r2(S_X2), Alu.add)

        nc.scalar.copy(w1(S_X4), w1(S_XV2))
        CP(w1(S_X4), m1(M_LD), w1(S_XV1))
        CP(w1(S_X4), m1(M_KD), w1(S_U))
        TT(w1(S_ACCM), gg(2, 2), w1(S_X4), Alu.add)

        nc.scalar.activation(
            w1(S_res), w1(S_ACCM), Act.Relu, bias=bias_sb[:, 3:4], scale=-0.5)
        nc.sync.dma_start(res, w1(S_res))

    nc.compile()
    return nc


def _host_prep(x, w, b):
    """Build per-core input maps."""
    import ml_dtypes

    x = np.ascontiguousarray(np.asarray(x, np.float32))
    w = np.asarray(w, np.float32)
    b = np.asarray(b, np.float32)

    w2m = np.zeros((C, 24), np.float32)
    for j in range(K):
        for f in range(F):
            w2m[:, j * 8 + f] = -2.0 * w[j, :, f]
    wh = w2m.astype(ml_dtypes.bfloat16)
    wlo = (w2m - wh.astype(np.float32)).astype(ml_dtypes.bfloat16)
    wcat = np.concatenate([wh, wlo], axis=1)              # [C, 48] bf16

    nW = (w ** 2).sum(1)                                  # [K, F]
    biasc = np.zeros((C, 4), np.float32)
    for q in range(NQ):
        for f in range(F):
            for j in range(K):
                biasc[q * 8 + f, j] = nW[j, f]
            biasc[q * 8 + f, 3] = b[f]

    in_maps = []
    for r in range(NCORES):
        x4 = x[r * NB:(r + 1) * NB]                       # [NB,T,C]
        flat = x4.reshape(TL, C)
        xT = np.ascontiguousarray(flat.T)                 # [C, TL] fp32
        xhh = xT.astype(ml_dtypes.bfloat16)
        xll = (xT - xhh.astype(np.float32)).astype(ml_dtypes.bfloat16)
        nS = np.einsum("tc,tc->t", flat, flat).astype(np.float32)
        nsr = np.ones((C, 3 * JW), np.float32)
        block = np.ones((NQ * 8, JW), np.float32)
        for q in range(NQ):
            lo = q * FD
            hi = min(TL, lo + FD + 2)
            block[q * 8:(q + 1) * 8, 0:hi - lo] = nS[lo:hi][None, :]
        for j in range(K):
            nsr[:, j * JW:(j + 1) * JW] = block
        in_maps.append({
            "xh": xhh, "xl": xll, "wcat": wcat, "nsr": nsr, "biasc": biasc,
        })
    return in_maps


def _assemble(results):
    out = np.empty((B, P, F), np.float32)
    for r in range(NCORES):
        resr = results[r]["res"]                          # [128, 512]
        arr = resr.reshape(NQ, 8, FD)                     # [q, f, p_lo]
        for f in range(F):
            series = arr[:, f, :].reshape(TL).reshape(NB, T)
            out[r * NB:(r + 1) * NB, :, f] = series[:, :P]
    return out


def kernel(x, w, b):
    from concourse.bass_utils import run_bass_kernel_spmd

    if "nc" not in _cache:
        _cache["nc"] = _build_program()
    nc = _cache["nc"]
    in_maps = _host_prep(x, w, b)
    out = run_bass_kernel_spmd(nc, in_maps, core_ids=list(range(NCORES)))
    return _assemble(out.results)


if __name__ == "__main__":
    rng = np.random.default_rng(0)
    x = rng.standard_normal((B, T, C), dtype=np.float32)
    w = (rng.standard_normal((K, C, F)) * 0.1).astype(np.float32)
    b = np.zeros((F,), np.float32)
    o = kernel(x, w, b)
    print("kernel ran, out shape", o.shape, float(np.abs(o).sum()))
